# revision 4
# baseline (speedup 1.0000x reference)
"""Trainium2 Bass kernel for ConvNet forward (conv7x7s3 -> sq -> fc -> sq -> fc).

Strategy: pure data parallel over 8 NeuronCores (2048 samples each).
The conv is lowered to a block-sparse dense matrix A [324, 2883] applied via
fp16 matmuls with batch as the moving free dim (N=512). Features are packed
row-major (r, ci, w) so each 3-output-row group covers a contiguous feature
span: 10 contraction chunks of 128 per group, 30 conv matmuls per tile.
Host pre-transposes x to feature-major fp16 layout; fc1 weights are permuted
to match the conv-output row grouping; the [10, 2048] per-core output is
transposed back to [B, 10] on the host.
"""

import numpy as np

for _p in ("/opt/trn_rl_repo", "/root/.axon_site/_ro/trn_rl_repo"):
    try:
        import concourse  # noqa: F401
        break
    except ImportError:
        import sys
        if _p not in sys.path:
            sys.path.insert(0, _p)

# network constants
KERNEL, STRIDE = 7, 3
C_IN, C_OUT = 3, 4
HIDDEN, OUTPUT = 64, 10
H_OUT = 9                      # (32-7)//3 + 1
B_TOT, N_CORES = 16384, 8
B_CORE = B_TOT // N_CORES      # 2048
N_TILE = 512                   # batch tile (matmul moving free dim)
T_TILES = B_CORE // N_TILE     # 4
M_GROUP = 3 * C_OUT * H_OUT    # 108 outputs per i-group (3 rows x 4 ch x 9 cols)

# Only 31x31 of each 32x32 input image is read by the conv (stride 3, k=7).
# Pack the 3*31*31 = 2883 used features ROW-MAJOR (r, ci, w) so that i-group g
# (output rows 3g..3g+2, needing input rows 9g..9g+12) covers the contiguous
# packed span [837g, 837g+1209) -> exactly 10 chunks of 128 per group.
ROW_F = C_IN * 31              # 93 features per input row
USED_IDX = np.array([ci * 1024 + r * 32 + w
                     for r in range(31) for ci in range(C_IN) for w in range(31)],
                    np.int64)
N_USED = len(USED_IDX)         # 2883
K_CHUNKS = 23
F_PACK = K_CHUNKS * 128        # 2944

GROUP_CHUNKS = [list(range(9 * g * ROW_F // 128,
                           (9 * g * ROW_F + 13 * ROW_F - 1) // 128 + 1))
                for g in range(3)]
PAIRS = [(g, k) for g in range(3) for k in GROUP_CHUNKS[g]]
N_PAIRS = len(PAIRS)           # 30


def _build_nc(repeat=1, mode="full"):
    import concourse.bacc as bacc
    import concourse.mybir as mybir
    from concourse.tile import TileContext

    F16 = mybir.dt.float16
    F32 = mybir.dt.float32
    AF = mybir.ActivationFunctionType

    nc = bacc.Bacc()
    # partition-major pack: [t, p, c, n] so each partition's DMA read is a
    # single contiguous run
    xT = nc.declare_dram_parameter(
        "xT", [T_TILES, 128, K_CHUNKS, N_TILE], F16, isOutput=False)
    ATp = nc.declare_dram_parameter("ATp", [128, N_PAIRS * M_GROUP], F16, isOutput=False)
    FC1 = nc.declare_dram_parameter("FC1", [M_GROUP, 3 * HIDDEN], F16, isOutput=False)
    FC2 = nc.declare_dram_parameter("FC2", [HIDDEN, OUTPUT], F16, isOutput=False)
    B1 = nc.declare_dram_parameter("B1", [HIDDEN, 1], F32, isOutput=False)
    B2 = nc.declare_dram_parameter("B2", [OUTPUT, 1], F32, isOutput=False)
    OUT = nc.declare_dram_parameter("OUT", [OUTPUT, B_CORE], F32, isOutput=True)

    with TileContext(nc) as tc:
        with tc.tile_pool(name="wpool", bufs=1) as wpool, \
             tc.tile_pool(name="xpool", bufs=4) as xpool, \
             tc.tile_pool(name="ypool", bufs=8) as ypool, \
             tc.tile_pool(name="opool", bufs=1) as opool, \
             tc.tile_pool(name="psy", bufs=4, space="PSUM") as psy, \
             tc.tile_pool(name="psh", bufs=2, space="PSUM") as psh, \
             tc.tile_pool(name="pso", bufs=2, space="PSUM") as pso:

            ats = wpool.tile([128, N_PAIRS * M_GROUP], F16, tag="ats")
            fc1t = wpool.tile([M_GROUP, 3 * HIDDEN], F16, tag="fc1t")
            fc2t = wpool.tile([HIDDEN, OUTPUT], F16, tag="fc2t")
            b1t = wpool.tile([HIDDEN, 1], F32, tag="b1t")
            b2t = wpool.tile([OUTPUT, 1], F32, tag="b2t")
            nc.sync.dma_start(out=ats, in_=ATp[:, :])
            nc.sync.dma_start(out=fc1t, in_=FC1[:, :])
            nc.sync.dma_start(out=fc2t, in_=FC2[:, :])
            nc.sync.dma_start(out=b1t, in_=B1[:, :])
            nc.sync.dma_start(out=b2t, in_=B2[:, :])
            outsb = opool.tile([OUTPUT, B_CORE], F32, tag="outsb")
            if mode == "dma":
                nc.gpsimd.memset(outsb, 0.0)

            if mode == "compute":
                xt_fixed = xpool.tile([128, K_CHUNKS, N_TILE], F16, tag="xt")
                nc.sync.dma_start(out=xt_fixed, in_=xT[0])

            # FC stage runs one tile behind the conv stage so the PE's
            # in-order queue never waits on Activation results: per tile the
            # PE streams [conv g0 | fc1(prev) | conv g1 | conv g2 | fc2(prev)]
            # while Act produces y2/h2 strictly ahead of their PE consumers.
            def emit_fc1(py2):
                hp = psh.tile([HIDDEN, N_TILE], F32, tag="psh")
                for g in range(3):
                    nc.tensor.matmul(
                        hp,
                        fc1t[:, g * HIDDEN:(g + 1) * HIDDEN],
                        py2[g],
                        start=(g == 0),
                        stop=(g == 2),
                    )
                return hp

            def emit_h2(hp):
                h2 = ypool.tile([HIDDEN, N_TILE], F16, tag="h2")
                nc.scalar.activation(h2, hp, AF.Square, bias=b1t)
                return h2

            def emit_fc2(h2, pt):
                op = pso.tile([OUTPUT, N_TILE], F32, tag="pso")
                nc.tensor.matmul(op, fc2t, h2, start=True, stop=True)
                nc.scalar.activation(
                    outsb[:, pt * N_TILE:(pt + 1) * N_TILE], op, AF.Identity,
                    bias=b2t,
                )

            pend = None                    # (y2 list, tile idx) awaiting fc
            for _rep in range(repeat):
                for t in range(T_TILES):
                    if mode == "compute":
                        xt = xt_fixed
                    else:
                        xt = xpool.tile([128, K_CHUNKS, N_TILE], F16, tag="xt")
                        nc.sync.dma_start(out=xt, in_=xT[t])
                    if mode == "dma":
                        continue
                    y2 = []
                    hp_prev = None
                    pcnt = 0
                    for g in range(3):
                        ps = psy.tile([M_GROUP, N_TILE], F32, tag="psy")
                        ks = GROUP_CHUNKS[g]
                        for idx, k in enumerate(ks):
                            nc.tensor.matmul(
                                ps,
                                ats[:, pcnt * M_GROUP:(pcnt + 1) * M_GROUP],
                                xt[:, k, :],
                                start=(idx == 0),
                                stop=(idx == len(ks) - 1),
                            )
                            pcnt += 1
                        if g == 0 and pend is not None:
                            hp_prev = emit_fc1(pend[0])
                        yt = ypool.tile([M_GROUP, N_TILE], F16, tag="y2")
                        nc.scalar.activation(yt, ps, AF.Square)
                        y2.append(yt)
                        if g == 1 and hp_prev is not None:
                            h2_prev = emit_h2(hp_prev)
                    if pend is not None:
                        emit_fc2(h2_prev, pend[1])
                    pend = (y2, t)
            if mode != "dma" and pend is not None:
                emit_fc2(emit_h2(emit_fc1(pend[0])), pend[1])
            nc.sync.dma_start(out=OUT[:, :], in_=outsb)
    nc.finalize()
    return nc


def _prep_weights(conv_w, fc1_w, fc1_b, fc2_w, fc2_b):
    # A[g, local, f]: dense conv matrix per i-group over packed features.
    # local = il*36 + c*9 + j  (i = 3g+il), packed f = r*93 + ci*31 + w
    Ap = np.zeros((3, M_GROUP, F_PACK), np.float32)
    for g in range(3):
        for il in range(3):
            i = 3 * g + il
            for c in range(C_OUT):
                for j in range(H_OUT):
                    row = il * 36 + c * 9 + j
                    for ci in range(C_IN):
                        for ki in range(KERNEL):
                            f0 = (3 * i + ki) * ROW_F + ci * 31 + 3 * j
                            Ap[g, row, f0:f0 + KERNEL] = conv_w[c, ci, ki, :]
    # pack the active [128, 108] transposed blocks side by side
    ATp = np.empty((128, N_PAIRS * M_GROUP), np.float16)
    for p, (g, k) in enumerate(PAIRS):
        ATp[:, p * M_GROUP:(p + 1) * M_GROUP] = Ap[g, :, 128 * k:128 * (k + 1)].T
    # fc1 columns permuted to our y-row order: global y row g*108+il*36+c*9+j
    # corresponds to reference flat index c*81 + (3g+il)*9 + j
    gg, ll, cc, jj = np.meshgrid(np.arange(3), np.arange(3), np.arange(C_OUT),
                                 np.arange(H_OUT), indexing="ij")
    orig = (cc * 81 + (3 * gg + ll) * 9 + jj).reshape(-1)
    fc1p = fc1_w[:, orig].T.astype(np.float16)        # [324, 64]
    FC1 = np.empty((M_GROUP, 3 * HIDDEN), np.float16)
    for g in range(3):
        FC1[:, g * HIDDEN:(g + 1) * HIDDEN] = fc1p[g * M_GROUP:(g + 1) * M_GROUP]
    FC2 = np.ascontiguousarray(fc2_w.T.astype(np.float16))  # [64, 10]
    B1 = np.ascontiguousarray(fc1_b.reshape(HIDDEN, 1).astype(np.float32))
    B2 = np.ascontiguousarray(fc2_b.reshape(OUTPUT, 1).astype(np.float32))
    return ATp, FC1, FC2, B1, B2


def _make_in_maps(x, ATp, FC1, FC2, B1, B2):
    in_maps = []
    xf = x.reshape(B_TOT, C_IN * 1024)
    for c in range(N_CORES):
        xs = xf[c * B_CORE:(c + 1) * B_CORE]
        xg = np.zeros((B_CORE, F_PACK), np.float16)
        xg[:, :N_USED] = xs[:, USED_IDX]
        xg = xg.reshape(T_TILES, N_TILE, K_CHUNKS, 128)
        xTc = np.ascontiguousarray(xg.transpose(0, 3, 2, 1))  # [4, 128, 23, 512]
        in_maps.append({"xT": xTc, "ATp": ATp, "FC1": FC1, "FC2": FC2,
                        "B1": B1, "B2": B2})
    return in_maps


def kernel(x, conv_w, fc1_w, fc1_b, fc2_w, fc2_b):
    from concourse.bass_utils import run_bass_kernel_spmd

    x = np.asarray(x, np.float32)
    ATp, FC1, FC2, B1, B2 = _prep_weights(
        np.asarray(conv_w, np.float32), np.asarray(fc1_w, np.float32),
        np.asarray(fc1_b, np.float32), np.asarray(fc2_w, np.float32),
        np.asarray(fc2_b, np.float32))

    in_maps = _make_in_maps(x, ATp, FC1, FC2, B1, B2)

    nc = _build_nc(repeat=1)
    res = run_bass_kernel_spmd(nc, in_maps, list(range(N_CORES)))
    out = np.empty((B_TOT, OUTPUT), np.float32)
    for c in range(N_CORES):
        out[c * B_CORE:(c + 1) * B_CORE] = res.results[c]["OUT"].T
    return out
